# revision 2
# baseline (speedup 1.0000x reference)
"""MinkowskiEngine transposed-conv + ReLU Bass kernel for 8 TRN2 cores.

Strategy (output-partitioned, no collectives):
  - core c owns output rows [c*RPC, (c+1)*RPC)
  - host buckets the K*N pairs per (core, feats-block, k); deals each bucket
    into 128-pair k-pure subchunks; 4 subchunks = one 512-pair window
  - device, per window: dma_gather 512 fp32 feats rows -> PE transpose per
    subchunk -> GEMM with W_k -> dma_scatter_add into one of NACC DRAM
    accumulators (Tile serializes same-tensor scatters; host guarantees
    distinct rows within each call)
  - final sweep: sum accumulators + bias, ReLU, write output
"""
import numpy as np
from contextlib import ExitStack

import concourse.bass as bass
import concourse.bacc as bacc
from concourse import mybir

P = 128
SUB_PER_WIN = 4
WIN_PAIRS = P * SUB_PER_WIN  # 512
NACC = 2
NQ = 4  # SWDGE queues


# ---------------------------------------------------------------------------
# host-side schedule construction
# ---------------------------------------------------------------------------

def build_schedule(in_map, out_map, n_in, n_out, n_cores, k_off, rng_seed=0):
    """Returns (sched, per_core) where sched is shared structure and per_core
    holds gather/scatter index arrays per core.

    sched: dict with rows_per_core, block_rows, n_blocks, sub2k[b] lists,
           n_win[b], feats_pad_rows, acc_rows
    """
    K, M = in_map.shape
    rpc = -(-n_out // n_cores)          # rows per core (output)
    rpc = -(-rpc // P) * P              # pad to 128
    block_rows = -(-n_in // 4)          # feats block size (int16 indexable)
    block_rows = -(-block_rows // P) * P
    n_blocks = -(-n_in // block_rows)
    assert block_rows <= 32768 and rpc + 256 <= 32768
    acc_rows = rpc + 256                # trailing trash rows
    trash = rpc                         # first trash row

    rng = np.random.default_rng(rng_seed)

    kk, ii = np.meshgrid(np.arange(K, dtype=np.int64), np.arange(M, dtype=np.int64),
                         indexing="ij")
    irow = in_map.astype(np.int64).ravel()
    orow = out_map.astype(np.int64).ravel()
    kfl = kk.ravel()
    core = orow // rpc
    blk = irow // block_rows
    irel = irow - blk * block_rows
    orel = orow - core * rpc

    # bucket counts per (core, block, k) -> shared subchunk counts
    S = np.zeros((n_blocks, K), np.int64)
    per_core_pairs = []
    for c in range(n_cores):
        m = core == c
        per_core_pairs.append((kfl[m], blk[m], irel[m], orel[m]))
        cnt = np.zeros((n_blocks, K), np.int64)
        np.add.at(cnt, (blk[m], kfl[m]), 1)
        S = np.maximum(S, -(-cnt // P))
    S = np.maximum(S, 1)

    sub2k = []          # per block: k id of each subchunk (shared)
    sub_ids = []        # per block: list of subchunk positions per k
    n_win = []
    for b in range(n_blocks):
        ks = []
        ids = [[] for _ in range(K)]
        # round-robin interleave so each (b,k) bucket spans many windows
        for r in range(int(S[b].max())):
            for k in range(K):
                if r < S[b, k]:
                    ids[k].append(len(ks))
                    ks.append(k)
        while len(ks) % SUB_PER_WIN:
            ks.append(0)                # full-padding subchunk
        sub2k.append(np.array(ks, np.int64))
        sub_ids.append(ids)
        n_win.append(len(ks) // SUB_PER_WIN)

    total_win = int(sum(n_win))
    sched = dict(rows_per_core=rpc, block_rows=block_rows, n_blocks=n_blocks,
                 sub2k=sub2k, n_win=n_win, total_win=total_win,
                 acc_rows=acc_rows, trash=trash,
                 feats_pad_rows=block_rows * n_blocks, K=K)

    # --- per-core assignment of pairs to (subchunk, slot) ---
    per_core = []
    for c in range(n_cores):
        kc, bc, ic, oc = per_core_pairs[c]
        gidx_cols = []
        sidx_cols = []
        for b in range(n_blocks):
            nsub = len(sub2k[b])
            gmat = np.zeros((nsub, P), np.int64)          # gather row (block-rel)
            smat = np.full((nsub, P), trash, np.int64)    # scatter row (core-rel)
            fill = np.zeros(nsub, np.int64)
            win_rows = [set() for _ in range(n_win[b])]   # rows per window
            mb = bc == b
            kb, ib, ob = kc[mb], ic[mb], oc[mb]
            # group by out-row; place each row's occurrences jointly so they
            # land in distinct windows (scatter-add calls must have unique
            # rows); most-constrained k (fewest subchunks) first
            order = np.lexsort((kb, ob))
            kb, ib, ob = kb[order], ib[order], ob[order]
            nS_of = np.array([len(sub_ids[b][k]) for k in range(K)])
            rot = np.zeros(K, np.int64)   # per-k rotating start for balance
            grp_start = np.concatenate(
                ([0], np.flatnonzero(ob[1:] != ob[:-1]) + 1, [len(ob)]))
            # sub -> list of (slot_index, row) for eviction lookups
            sub_rows = [dict() for _ in range(nsub)]

            def place(j, ro, k, allow_evict=True):
                ids = sub_ids[b][k]
                nS = len(ids)
                for probe in range(nS):
                    s = ids[int((rot[k] + probe) % nS)]
                    w = s // SUB_PER_WIN
                    if fill[s] < P and ro not in win_rows[w]:
                        gmat[s, fill[s]] = ib[j]
                        smat[s, fill[s]] = ro
                        sub_rows[s][ro] = int(fill[s])
                        fill[s] += 1
                        win_rows[w].add(ro)
                        rot[k] = (rot[k] + 1) % nS
                        return True
                if not allow_evict:
                    return False
                # one-level eviction: ro blocks every window with space; move
                # ro's earlier occurrence out of some window w to make room
                for probe in range(nS):
                    s = ids[int((rot[k] + probe) % nS)]
                    w = s // SUB_PER_WIN
                    if fill[s] >= P or ro not in win_rows[w]:
                        continue
                    # find ro's placement within window w
                    for s2 in range(w * SUB_PER_WIN,
                                    min((w + 1) * SUB_PER_WIN, nsub)):
                        if ro in sub_rows[s2]:
                            slot2 = sub_rows[s2][ro]
                            k2 = int(sub2k[b][s2])
                            ri2 = int(gmat[s2, slot2])
                            # tentatively remove and try to re-place elsewhere
                            win_rows[w].discard(ro)
                            last = fill[s2] - 1
                            mv_ro = int(smat[s2, last])
                            gmat[s2, slot2] = gmat[s2, last]
                            smat[s2, slot2] = smat[s2, last]
                            if mv_ro != trash and mv_ro in sub_rows[s2] \
                                    and sub_rows[s2][mv_ro] == last:
                                sub_rows[s2][mv_ro] = slot2
                            del sub_rows[s2][ro]
                            fill[s2] -= 1
                            ok2 = _place_raw(ri2, ro, k2)
                            if ok2:
                                # now place the original pair in s
                                gmat[s, fill[s]] = ib[j]
                                smat[s, fill[s]] = ro
                                sub_rows[s][ro] = int(fill[s])
                                fill[s] += 1
                                win_rows[w].add(ro)
                                return True
                            # undo removal
                            _undo_place(s2, ri2, ro, slot2, mv_ro)
                            win_rows[w].add(ro)
                            break
                return False

            def _place_raw(ri, ro, k):
                ids = sub_ids[b][k]
                nS = len(ids)
                for probe in range(nS):
                    s = ids[int((rot[k] + probe) % nS)]
                    w = s // SUB_PER_WIN
                    if fill[s] < P and ro not in win_rows[w]:
                        gmat[s, fill[s]] = ri
                        smat[s, fill[s]] = ro
                        sub_rows[s][ro] = int(fill[s])
                        fill[s] += 1
                        win_rows[w].add(ro)
                        rot[k] = (rot[k] + 1) % nS
                        return True
                return False

            def _undo_place(s2, ri, ro, slot2, mv_ro):
                last = fill[s2]
                if mv_ro != trash and mv_ro in sub_rows[s2] \
                        and sub_rows[s2][mv_ro] == slot2:
                    sub_rows[s2][mv_ro] = last
                gmat[s2, last] = gmat[s2, slot2]
                smat[s2, last] = smat[s2, slot2]
                gmat[s2, slot2] = ri
                smat[s2, slot2] = ro
                sub_rows[s2][ro] = slot2
                fill[s2] += 1

            # most-constrained rows (highest multiplicity) first
            sizes = np.diff(grp_start)
            for gi in np.argsort(-sizes, kind="stable"):
                lo, hi = grp_start[gi], grp_start[gi + 1]
                ro = int(ob[lo])
                occs = sorted(range(lo, hi), key=lambda j: nS_of[kb[j]])
                for j in occs:
                    ok = place(j, ro, int(kb[j]))
                    assert ok, ("unplaceable pair", c, b, int(kb[j]), ro)
            gidx_cols.append(gmat)
            sidx_cols.append(smat)
        gmat = np.concatenate(gidx_cols, 0)   # [total_sub, P]
        smat = np.concatenate(sidx_cols, 0)
        per_core.append((gmat, smat))

    # verify: within every window scatter rows (non-trash) distinct
    for c in range(n_cores):
        gmat, smat = per_core[c]
        off = 0
        for b in range(n_blocks):
            nsub = len(sub2k[b])
            sm = smat[off:off + nsub].reshape(-1, SUB_PER_WIN * P)
            for w in range(sm.shape[0]):
                rows = sm[w][sm[w] != trash]
                assert len(rows) == len(np.unique(rows)), (c, b, w)
            off += nsub
    return sched, per_core


def pack_idx16(mat):
    """[nsub, P] int -> wire layout [128, nsub//4 * 32] int16 per window.

    Window pair j (0..511) idx consumed from [j%16, j//16] of a [128, 32]
    int16 tile; pair j = subchunk j//128, slot j%128.
    """
    nsub = mat.shape[0]
    assert nsub % SUB_PER_WIN == 0
    nwin = nsub // SUB_PER_WIN
    out = np.zeros((P, nwin * 32), np.int16)
    w = mat.reshape(nwin, SUB_PER_WIN * P).astype(np.int16)   # [nwin, 512] j-order
    for n in range(nwin):
        out[:, n * 32:(n + 1) * 32] = np.tile(w[n].reshape(-1, 16).T, (8, 1))
    return out


# ---------------------------------------------------------------------------
# device program
# ---------------------------------------------------------------------------

def build_program_raw(sched, c_in=256, c_out=128):
    """Raw-bass (manual semaphores) SPMD program, deep-pipelined.

    gpsimd prefetches gathers DEPTH windows ahead of the scatters so SWDGE
    descriptor generation, PE, ACT/DVE copies and SDMA transfers all overlap.
    SWDGE sem lane l is only ever used with queue l%4 (hardware lock rule).
    """
    K = sched["K"]
    n_blocks = sched["n_blocks"]
    block_rows = sched["block_rows"]
    acc_rows = sched["acc_rows"]
    rpc = sched["rows_per_core"]
    total_win = sched["total_win"]
    NSEM_G = 8
    NG, NTT, NCS, NSW, LAG, DEPTH = 8, 4, 8, 4, 2, 6

    nc = bacc.Bacc("TRN2", target_bir_lowering=False, debug=False,
                   num_swdge_queues=NQ)
    feats = nc.dram_tensor("feats", [sched["feats_pad_rows"], c_in],
                           mybir.dt.float32, kind="ExternalInput").ap()
    wmat = nc.dram_tensor("wmat", [P, K * 2 * c_out], mybir.dt.float32,
                          kind="ExternalInput").ap()
    bias_b = nc.dram_tensor("bias_b", [P, c_out], mybir.dt.float32,
                            kind="ExternalInput").ap()
    ident_in = nc.dram_tensor("ident_in", [P, P], mybir.dt.float32,
                              kind="ExternalInput").ap()
    gidx = nc.dram_tensor("gidx", [P, total_win * 32], mybir.dt.int16,
                          kind="ExternalInput").ap()
    sidx = nc.dram_tensor("sidx", [P, total_win * 32], mybir.dt.int16,
                          kind="ExternalInput").ap()
    accs = [nc.dram_tensor(f"acc{a}", [acc_rows, c_out], mybir.dt.float32,
                           kind="ExternalOutput").ap() for a in range(NACC)]
    out = nc.dram_tensor("out", [rpc, c_out], mybir.dt.float32,
                         kind="ExternalOutput").ap()

    sub_k = []
    win_block = []
    for b in range(n_blocks):
        win_block += [b] * sched["n_win"][b]
        sub_k.extend(int(x) for x in sched["sub2k"][b])
    U = len(sub_k)
    W = total_win
    assert U == SUB_PER_WIN * W

    # PE op-pair sequence; pe_sem value == index+1 once that pair completes
    pe_seq = []
    for u in range(U + 2):
        if u < U:
            pe_seq.append(("T", u))
        if u >= 2:
            pe_seq.append(("MM", u - 2))
    pe_pos = {e: i + 1 for i, e in enumerate(pe_seq)}

    n_sweep = rpc // P
    n_seed = acc_rows // P
    LOAD_TOTAL = 16 * (4 + n_seed)

    with ExitStack() as stack:
        block = stack.enter_context(nc.Block())
        load_sem = stack.enter_context(nc.semaphore("load"))
        bias_ld = stack.enter_context(nc.semaphore("bias_ld"))
        pe_sem = stack.enter_context(nc.semaphore("pe"))
        act_sem = stack.enter_context(nc.semaphore("act"))
        dve_sem = stack.enter_context(nc.semaphore("dve"))
        swp_sems = [stack.enter_context(nc.semaphore(f"swp{i}")) for i in range(NSW)]
        swo_sems = [stack.enter_context(nc.semaphore(f"swo{i}")) for i in range(NSW)]
        g_sems = [stack.enter_context(nc.semaphore(f"g{i}")) for i in range(NSEM_G)]
        s_sems = [stack.enter_context(nc.semaphore(f"s{i}")) for i in range(NQ)]

        w_sb = stack.enter_context(
            nc.sbuf_tensor("w_sb", [P, K * 2 * c_out], mybir.dt.float32))
        bias_sb = stack.enter_context(
            nc.sbuf_tensor("bias_sb", [P, c_out], mybir.dt.float32))
        ident = stack.enter_context(
            nc.sbuf_tensor("ident", [P, P], mybir.dt.float32))
        gi_sb = stack.enter_context(
            nc.sbuf_tensor("gi_sb", [P, total_win * 32], mybir.dt.int16))
        si_sb = stack.enter_context(
            nc.sbuf_tensor("si_sb", [P, total_win * 32], mybir.dt.int16))
        g_sb = stack.enter_context(
            nc.sbuf_tensor("g_sb", [P, NG, SUB_PER_WIN, c_in], mybir.dt.float32))
        tt_sb = stack.enter_context(
            nc.sbuf_tensor("tt_sb", [P, NTT, 2, P], mybir.dt.float32))
        cs_sb = stack.enter_context(
            nc.sbuf_tensor("cs_sb", [P, NCS, SUB_PER_WIN, c_out], mybir.dt.float32))
        sw_sb = stack.enter_context(
            nc.sbuf_tensor("sw_sb", [P, NSW, 2, c_out], mybir.dt.float32))
        r1_sb = stack.enter_context(
            nc.sbuf_tensor("r1_sb", [P, NSW, c_out], mybir.dt.float32))
        r_sb = stack.enter_context(
            nc.sbuf_tensor("r_sb", [P, NSW, c_out], mybir.dt.float32))
        # one PSUM bank (512 fp32/partition) per tile: PE-write and
        # ACT/DVE-read must never share a bank
        tps = stack.enter_context(
            nc.psum_tensor("tps", [P, 3, 2, 512], mybir.dt.float32))  # banks 0-5
        cps = stack.enter_context(
            nc.psum_tensor("cps", [P, 2, 512], mybir.dt.float32))     # banks 6-7

        @block.sync
        def _(sy):
            sy.dma_start(out=w_sb[:], in_=wmat[:]).then_inc(load_sem, 16)
            sy.dma_start(out=bias_sb[:], in_=bias_b[:]).then_inc(bias_ld, 16)
            sy.dma_start(out=ident[:], in_=ident_in[:]).then_inc(load_sem, 16)
            sy.dma_start(out=gi_sb[:], in_=gidx[:]).then_inc(load_sem, 16)
            sy.dma_start(out=si_sb[:], in_=sidx[:]).then_inc(load_sem, 16)
            sy.wait_ge(bias_ld, 16)
            # seed acc0 with bias so the final sweep is add + relu only
            for t in range(n_seed):
                sy.dma_start(out=accs[0][t * P:(t + 1) * P, :],
                             in_=bias_sb[:]).then_inc(load_sem, 16)
            for l in range(NQ):
                cnt = sum(1 for w in range(W) if w % NQ == l)
                sy.wait_ge(s_sems[l], 16 * cnt)
            for t in range(n_sweep + LAG):
                if t < n_sweep:
                    slot = t % NSW
                    if t >= NSW:
                        sy.wait_ge(dve_sem, 2 * U + (t - NSW + 1))
                    sy.dma_start(out=sw_sb[:, slot, 0, :],
                                 in_=accs[0][t * P:(t + 1) * P, :]
                                 ).then_inc(swp_sems[slot], 16)
                    sy.dma_start(out=sw_sb[:, slot, 1, :],
                                 in_=accs[1][t * P:(t + 1) * P, :]
                                 ).then_inc(swp_sems[slot], 16)
                if t >= LAG:
                    tt = t - LAG
                    sy.wait_ge(act_sem, U + tt + 1)
                    sy.dma_start(out=out[tt * P:(tt + 1) * P, :],
                                 in_=r_sb[:, tt % NSW, :]
                                 ).then_inc(swo_sems[tt % NSW], 16)

        @block.tensor
        def _(pe):
            pe.wait_ge(load_sem, LOAD_TOTAL)
            pe.wait_ge(bias_ld, 16)
            for (op, u) in pe_seq:
                w = u // SUB_PER_WIN
                s = u % SUB_PER_WIN
                if op == "T":
                    if s == 0:
                        pe.wait_ge(g_sems[w % NSEM_G], 16 * (w // NSEM_G + 1))
                    if u >= 3:
                        # tps bank reuse: both TT copies of u-3 done
                        x = u - 3
                        pe.wait_ge(act_sem, x + 1)
                        pe.wait_ge(dve_sem, 2 * x - 1 if x >= 2 else x + 1)
                    pe.transpose(out=tps[:, u % 3, 0, :P],
                                 in_=g_sb[:, w % NG, s, 0:P], identity=ident[:])
                    pe.transpose(out=tps[:, u % 3, 1, :P],
                                 in_=g_sb[:, w % NG, s, P:2 * P], identity=ident[:]
                                 ).then_inc(pe_sem, 1)
                else:
                    k = sub_k[u]
                    pe.wait_ge(act_sem, u + 1)        # TT_lo(u) written
                    # TT_hi(u) done (and for u>=2 also contrib(u-2) -> cps free)
                    pe.wait_ge(dve_sem, 2 * u if u >= 2 else u + 1)
                    pe.matmul(out=cps[:, u % 2, :c_out],
                              lhsT=tt_sb[:, u % NTT, 0, :],
                              rhs=w_sb[:, (k * 2) * c_out:(k * 2 + 1) * c_out],
                              start=True, stop=False)
                    pe.matmul(out=cps[:, u % 2, :c_out],
                              lhsT=tt_sb[:, u % NTT, 1, :],
                              rhs=w_sb[:, (k * 2 + 1) * c_out:(k * 2 + 2) * c_out],
                              start=False, stop=True).then_inc(pe_sem, 1)

        @block.scalar
        def _(sc):
            sc.wait_ge(load_sem, LOAD_TOTAL)
            sc.wait_ge(bias_ld, 16)
            for u in range(U):
                sc.wait_ge(pe_sem, pe_pos[("T", u)])
                if u >= NTT:
                    sc.wait_ge(pe_sem, pe_pos[("MM", u - NTT)])  # tt slot reuse
                sc.copy(out=tt_sb[:, u % NTT, 0, :], in_=tps[:, u % 3, 0, :P]
                        ).then_inc(act_sem, 1)
            for t in range(n_sweep):
                slot = t % NSW
                sc.wait_ge(dve_sem, 2 * U + t + 1)
                if t >= NSW:
                    sc.wait_ge(swo_sems[slot], 16 * (t // NSW))  # r_sb reuse
                sc.activation(out=r_sb[:, slot, :], in_=r1_sb[:, slot, :],
                              func=mybir.ActivationFunctionType.Relu
                              ).then_inc(act_sem, 1)

        @block.vector
        def _(ve):
            ve.wait_ge(load_sem, LOAD_TOTAL)
            ve.wait_ge(bias_ld, 16)
            for u in range(U):
                w = u // SUB_PER_WIN
                s = u % SUB_PER_WIN
                ve.wait_ge(pe_sem, pe_pos[("T", u)])
                if u >= NTT:
                    ve.wait_ge(pe_sem, pe_pos[("MM", u - NTT)])  # tt slot reuse
                ve.tensor_copy(out=tt_sb[:, u % NTT, 1, :], in_=tps[:, u % 3, 1, :P]
                               ).then_inc(dve_sem, 1)
                if u >= 2:
                    u2 = u - 2
                    w2 = u2 // SUB_PER_WIN
                    s2 = u2 % SUB_PER_WIN
                    ve.wait_ge(pe_sem, pe_pos[("MM", u2)])
                    if s2 == 0 and w2 >= NCS:
                        lw = w2 - NCS                 # cs_sb slot reuse
                        ve.wait_ge(s_sems[lw % NQ], 16 * (lw // NQ + 1))
                    ve.tensor_copy(out=cs_sb[:, w2 % NCS, s2, :],
                                   in_=cps[:, u2 % 2, :c_out]).then_inc(dve_sem, 1)
            for u2 in range(U - 2, U):
                w2 = u2 // SUB_PER_WIN
                s2 = u2 % SUB_PER_WIN
                ve.wait_ge(pe_sem, pe_pos[("MM", u2)])
                ve.tensor_copy(out=cs_sb[:, w2 % NCS, s2, :],
                               in_=cps[:, u2 % 2, :c_out]).then_inc(dve_sem, 1)
            for t in range(n_sweep):
                slot = t % NSW
                ve.wait_ge(swp_sems[slot], 32 * (t // NSW + 1))
                if t >= NSW:
                    ve.wait_ge(act_sem, U + (t - NSW) + 1)  # r1 slot reuse
                ve.tensor_add(out=r1_sb[:, slot, :], in0=sw_sb[:, slot, 0, :],
                              in1=sw_sb[:, slot, 1, :]).then_inc(dve_sem, 1)

        @block.gpsimd
        def _(gp):
            from concourse.library_config import mlp
            gp.load_library(mlp)
            gp.wait_ge(load_sem, LOAD_TOTAL)
            gp.wait_ge(bias_ld, 16)
            for w in range(W + DEPTH):
                if w < W:
                    b = win_block[w]
                    if w >= NG:
                        lu = (w - NG) * SUB_PER_WIN + (SUB_PER_WIN - 1)
                        gp.wait_ge(pe_sem, pe_pos[("T", lu)])  # g slot reuse
                    gp.dma_gather(
                        g_sb[:, w % NG, :, :],
                        feats[b * block_rows:(b + 1) * block_rows, :],
                        gi_sb[:, w * 32:(w + 1) * 32],
                        WIN_PAIRS, WIN_PAIRS, c_in,
                        transpose=False, queue_num=w % NQ,
                    ).then_inc(g_sems[w % NSEM_G], 16)
                if w >= DEPTH:
                    ws = w - DEPTH
                    lu = ws * SUB_PER_WIN + (SUB_PER_WIN - 1)
                    # contrib copies of window ws complete
                    if lu <= U - 3:
                        gp.wait_ge(dve_sem, 2 * lu + 4)
                    else:
                        gp.wait_ge(dve_sem, 2 * U - (U - 1 - lu))
                    if ws >= NACC:
                        pa = ws - NACC   # same-acc predecessor done (RMW order)
                        gp.wait_ge(s_sems[pa % NQ], 16 * (pa // NQ + 1))
                    gp.dma_scatter_add(
                        accs[ws % NACC], cs_sb[:, ws % NCS, :, :],
                        si_sb[:, ws * 32:(ws + 1) * 32],
                        WIN_PAIRS, WIN_PAIRS, c_out,
                        queue_num=ws % NQ,
                    ).then_inc(s_sems[ws % NQ], 16)

    nc.compile()
    return nc


def make_inputs(feats, weight, bias, sched, per_core):
    n_in, c_in = feats.shape
    K, _, c_out = weight.shape
    fp = np.zeros((sched["feats_pad_rows"], c_in), np.float32)
    fp[:n_in] = feats
    # wmat layout: [p, (k*2+h)*c_out + j] = weight[k, h*128 + p, j]
    wm = np.ascontiguousarray(
        weight.astype(np.float32).reshape(K, 2, P, c_out).transpose(2, 0, 1, 3)
    ).reshape(P, K * 2 * c_out)
    bb = np.tile(bias.astype(np.float32)[None, :], (P, 1))
    ident = np.eye(P, dtype=np.float32)
    in_maps = []
    for (gmat, smat) in per_core:
        in_maps.append(dict(feats=fp, wmat=wm, bias_b=bb, ident_in=ident,
                            gidx=pack_idx16(gmat), sidx=pack_idx16(smat)))
    return in_maps


# ---------------------------------------------------------------------------
# harness entry point
# ---------------------------------------------------------------------------

N_CORES = 8
_CACHE = {}


def kernel(feats, weight, bias, in_map, out_map, n_out):
    """Full-input entry: shards across 8 NeuronCores internally."""
    from concourse.bass_utils import run_bass_kernel_spmd

    feats = np.asarray(feats, dtype=np.float32)
    weight = np.asarray(weight, dtype=np.float32)
    bias = np.asarray(bias, dtype=np.float32)
    in_map = np.asarray(in_map)
    out_map = np.asarray(out_map)
    n_out = int(n_out)
    n_in = feats.shape[0]
    K = weight.shape[0]

    sched, per_core = build_schedule(in_map, out_map, n_in, n_out, N_CORES, K)
    in_maps = make_inputs(feats, weight, bias, sched, per_core)

    key = (n_in, n_out, K, sched["total_win"])
    nc = _CACHE.get(key)
    if nc is None:
        nc = build_program_raw(sched)
        _CACHE[key] = nc

    res = run_bass_kernel_spmd(nc, in_maps, list(range(N_CORES)))
    rpc = sched["rows_per_core"]
    got = np.concatenate([res.results[c]["out"][:rpc] for c in range(N_CORES)], 0)
    return np.ascontiguousarray(got[:n_out])
